# revision 5
# baseline (speedup 1.0000x reference)
"""Trainium2 Bass kernel v2 for nn_AttentionModule (gated-SE + attention pooling).

Math (per reference):
  att = tanh(relu(x@w1+b1)@w2+b2); x2 = (1+att)*x = 2*sigmoid(2*(pre+b2))*x
  mean = segment_mean(x2, batch); tg = tanh(mean @ W)
  coef = sigmoid(sum(x2 * tg[batch], -1)); out = segment_sum(coef[:,None]*x2, batch)

v2 strategy (driven by measured DMA behavior: per-call cost is dominated by
TOTAL bytes moved HBM<->SBUF at ~20 GB/s aggregate across the 8 cores;
compute engines are essentially free in that shadow):

- Slot-uniform layout: graphs are sorted by node count and binned into 8
  size groups of 256 graphs (32 per core per group). Group g pads every
  graph to S_g slots (max count in group, rounded to 16), so padding is
  ~3% instead of ~25%. All segment reductions become static-shape DVE
  3D reduces over [128, 32, S_g] -- no masks, no PE transposes.
- Per core: 8 windows (one per size group) of 32 graphs. x stored
  feature-major [128, cols] bf16 in DRAM (host pre-transposed), one big
  contiguous DMA per window.
- coef path: per-graph dots via matmul(lhsT=tg[:,g], rhs=x2[:, slots]) ->
  [1, S_g]; row of dots broadcast back to 128 partitions via a K=1 matmul
  with a ones row; sigmoid on [128,512] tiles; weighted x2 reduced per
  graph slots. Everything stays column-major (feature-major).
- ACT uses only Relu/Sigmoid/Copy (one table set; tanh is computed as
  2*sigmoid(2z)-1 with the affine on DVE) -- no table reloads.
- x2' = sigmoid(2z)*x = x2/2 is used on device; factor 2 folded into
  inv_counts, the coef sigmoid scale, and a final host-side doubling.
"""

from contextlib import ExitStack

import numpy as np

P = 128
D = 128
R = 32
G = 2048
NCORES = 8
WG = 32                  # graphs per window
NGRP = 8                 # size groups == windows per core
GPC = G // NCORES        # graphs per core = 256

_F32 = np.float32


def _bf16():
    import ml_dtypes
    return ml_dtypes.bfloat16


# ---------------------------------------------------------------- host prep

def _plan(batch):
    """Sorted-slot plan: graph order, group slot sizes, column offsets."""
    counts = np.bincount(batch, minlength=G).astype(np.int64)
    order = np.argsort(counts, kind="stable")      # rank -> graph id
    rank_of = np.empty(G, dtype=np.int64)
    rank_of[order] = np.arange(G)
    S = np.zeros(NGRP, dtype=np.int64)
    for g in range(NGRP):
        mx = int(counts[order[256 * g : 256 * (g + 1)]].max())
        S[g] = max(16, ((mx + 15) // 16) * 16)
    assert int(S.max()) <= 512, f"group slot size too large: {S}"
    off = np.zeros(NGRP + 1, dtype=np.int64)
    off[1:] = np.cumsum(WG * S)
    return counts, order, rank_of, S, off


def _quantize(x):
    """int8 linear quantization of x, round-to-nearest, symmetric scale."""
    x = x.astype(np.float32)
    step = np.float32(np.abs(x).max() / 127.0)
    if step == 0.0:
        step = np.float32(1.0)
    q = np.clip(np.round(x / step), -127, 127).astype(np.int8)
    return q, step


def _prep(x, batch, q8):
    counts, order, rank_of, S, off = _plan(batch)
    TOT = int(off[-1])

    cum = np.concatenate([[0], np.cumsum(counts)])
    rk = rank_of[batch]                      # per node: rank of its graph
    g = rk // 256
    within = rk % 256
    core = within // WG
    r = within % WG
    col = off[g] + r * S[g] + (np.arange(len(batch)) - cum[batch])

    xT = np.zeros((NCORES, P, TOT), dtype=np.int8)
    for c in range(NCORES):
        m = core == c
        buf = np.zeros((TOT, D), dtype=np.int8)
        buf[col[m]] = q8[m]
        xT[c] = buf.T

    # inv-counts (x2 = 2*x2' folded here), laid out in window-column order
    ic = np.zeros((NCORES, 1, GPC), dtype=_F32)
    gid = np.empty((NCORES, GPC), dtype=np.int64)   # (core, colidx) -> graph
    for c in range(NCORES):
        gcols = np.arange(GPC)
        gg = gcols // WG
        rr = gcols % WG
        ranks = 256 * gg + WG * c + rr
        gid[c] = order[ranks]
        ic[c, 0] = 2.0 / np.maximum(counts[gid[c]], 1)
    return xT, ic, gid, S, off, TOT


def _pack_params(fc_w1, fc_w2, W, fc_b1, fc_b2):
    bf16 = _bf16()
    # pkb (bf16): [w1s 0:32][w2s(4x tiled) 32:160][ones-row block 160:288]
    pkb = np.zeros((P, 288), dtype=bf16)
    pkb[:, 0:R] = fc_w1.astype(bf16)
    pkb[:, R : R + P] = np.tile(fc_w2, (4, 1)).astype(bf16)
    pkb[0, R + P : R + 2 * P] = bf16(1.0)
    # pkf (f32): [Ws 0:128][b1 128][b2 129][ones-row 130:258]
    pkf = np.zeros((P, 258), dtype=_F32)
    pkf[:, 0:P] = W
    pkf[:, P] = np.tile(fc_b1, 4)
    pkf[:, P + 1] = 2.0 * fc_b2
    pkf[0, P + 2 : P + 2 + P] = 1.0
    return pkb, pkf


# ---------------------------------------------------------------- program

def _build(S, step, use_b1=False, use_b2=False, probe=0):
    import concourse.bass as bass_mod
    import concourse.bacc as bacc
    import concourse.tile as tile
    from concourse import mybir
    from concourse.alu_op_type import AluOpType

    f32 = mybir.dt.float32
    bf = mybir.dt.bfloat16
    i8 = mybir.dt.int8
    AF = mybir.ActivationFunctionType
    S = [int(s) for s in S]
    off = [0]
    for s in S:
        off.append(off[-1] + WG * s)
    TOT = off[-1]

    nc = bacc.Bacc()
    xd = nc.dram_tensor("x", [P, TOT], i8, kind="ExternalInput")
    pkbd = nc.dram_tensor("pkb", [P, 288], bf, kind="ExternalInput")
    pkfd = nc.dram_tensor("pkf", [P, 258], f32, kind="ExternalInput")
    icd = nc.dram_tensor("ic", [1, GPC], f32, kind="ExternalInput")
    outd = nc.dram_tensor("out", [P, GPC], f32, kind="ExternalOutput")

    with tile.TileContext(nc) as tc, ExitStack() as ctx:
        sing = ctx.enter_context(tc.tile_pool(name="sing", bufs=1))
        xtp = ctx.enter_context(tc.tile_pool(name="xtp", bufs=4))
        xbp = ctx.enter_context(tc.tile_pool(name="xbp", bufs=2))
        x2p = ctx.enter_context(tc.tile_pool(name="x2p", bufs=2))
        wtp = ctx.enter_context(tc.tile_pool(name="wtp", bufs=2))
        drp = ctx.enter_context(tc.tile_pool(name="drp", bufs=1))
        hsp = ctx.enter_context(tc.tile_pool(name="hsp", bufs=3))
        sgp = ctx.enter_context(tc.tile_pool(name="sgp", bufs=3))
        mds = ctx.enter_context(tc.tile_pool(name="mds", bufs=4))
        # PSUM (8 banks): hpp 1 + app 2 + dpp 2 + cpp 2 + mpp 1
        hpp = ctx.enter_context(tc.tile_pool(name="hpp", bufs=1, space="PSUM"))
        app = ctx.enter_context(tc.tile_pool(name="app", bufs=1, space="PSUM"))
        dpp = ctx.enter_context(tc.tile_pool(name="dpp", bufs=2, space="PSUM"))
        cpp = ctx.enter_context(tc.tile_pool(name="cpp", bufs=1, space="PSUM"))
        mpp = ctx.enter_context(tc.tile_pool(name="mpp", bufs=1, space="PSUM"))

        pkb = sing.tile([P, 288], bf)
        nc.sync.dma_start(out=pkb, in_=pkbd[:, :])
        pkf = sing.tile([P, 258], f32)
        nc.sync.dma_start(out=pkf, in_=pkfd[:, :])
        w1s = pkb[:, 0:R]
        w2s = pkb[:, R : R + P]
        onesb = pkb[0:1, R + P : R + 2 * P]          # [1,128] bf16 ones row
        Ws = pkf[:, 0:P]
        b1s = pkf[:, P : P + 1]
        b2s = pkf[:, P + 1 : P + 2]
        onesf = pkf[0:1, P + 2 : P + 2 + P]          # [1,128] f32 ones row

        icrow = sing.tile([1, GPC], f32)
        nc.sync.dma_start(out=icrow, in_=icd[:, :])
        icp = mpp.tile([P, GPC], f32, tag="mp")
        nc.tensor.matmul(icp, lhsT=onesf, rhs=icrow, start=True, stop=True)
        icb = sing.tile([P, GPC], f32)
        nc.vector.tensor_copy(icb, icp)

        outacc = sing.tile([P, GPC], f32)

        for g in range(NGRP):
            Sg = S[g]
            WIN = WG * Sg
            NC = WIN // 512          # chunks of 512 (Sg % 16 == 0)
            xw = xtp.tile([P, WIN], i8, tag="x")
            nc.sync.dma_start(out=xw, in_=xd[:, off[g] : off[g] + WIN])
            x2w = x2p.tile([P, WIN], bf, tag="x2")
            xbw = xbp.tile([P, WIN], bf, tag="xb")
            nc.scalar.activation(xbw, xw, AF.Copy, scale=float(step))
            if probe >= 2:      # DMA+decode probe
                nc.vector.reduce_sum(outacc[:, WG * g : WG * (g + 1)],
                                     xbw.rearrange("p (r s) -> p r s", s=Sg),
                                     axis=mybir.AxisListType.X)
                continue

            # ---- phase A: x2' = sigmoid(2(pre+b2)) * x, chunkwise
            for q4 in range(0, NC, 4):
                nq = min(4, NC - q4)
                hps = hpp.tile([P, 512], f32, tag="h")
                for sb in range(nq):
                    c0 = (q4 + sb) * 512
                    nc.tensor.matmul(hps[32 * sb : 32 * sb + 32, :],
                                     lhsT=w1s, rhs=xbw[:, c0 : c0 + 512],
                                     start=True, stop=True,
                                     tile_position=(0, 32 * sb))
                hs = hsp.tile([P, 512], bf, tag="hs")
                nc.scalar.activation(hs[: 32 * nq, :], hps[: 32 * nq, :],
                                     AF.Relu, bias=b1s if use_b1 else 0.0)
                sb = 0
                while sb < nq:
                    w = 1024 if sb + 1 < nq else 512
                    c0 = (q4 + sb) * 512
                    att = app.tile([P, 1024], f32, tag="att")
                    for j in range(w // 512):
                        nc.tensor.matmul(att[:, 512 * j : 512 * (j + 1)],
                                         lhsT=w2s[32 * (sb + j) : 32 * (sb + j) + 32, :],
                                         rhs=hs[32 * (sb + j) : 32 * (sb + j) + 32, :],
                                         start=True, stop=True,
                                         tile_position=(32 * (sb + j), 0))
                    sg = sgp.tile([P, 1024], bf, tag="sg")
                    nc.scalar.activation(sg[:, :w], att[:, :w], AF.Sigmoid,
                                         bias=b2s if use_b2 else 0.0, scale=2.0)
                    nc.vector.tensor_tensor(x2w[:, c0 : c0 + w], sg[:, :w],
                                            xbw[:, c0 : c0 + w],
                                            op=AluOpType.mult)
                    sb += w // 512

            if probe == 1:      # DMA + MLP probe: reduce x2 straight to out
                nc.vector.reduce_sum(outacc[:, WG * g : WG * (g + 1)],
                                     x2w.rearrange("p (r s) -> p r s", s=Sg),
                                     axis=mybir.AxisListType.X)
                continue

            # ---- phase B: mean -> tg (= tanh via 2*sigmoid(2z)-1)
            seg = mds.tile([P, WG], f32, tag="seg")
            nc.vector.reduce_sum(seg, x2w.rearrange("p (r s) -> p r s", s=Sg),
                                 axis=mybir.AxisListType.X)
            mean = mds.tile([P, WG], f32, tag="mean")
            nc.vector.tensor_tensor(mean, seg, icb[:, WG * g : WG * (g + 1)],
                                    op=AluOpType.mult)
            tgps = mpp.tile([P, GPC], f32, tag="mp")
            nc.tensor.matmul(tgps[:, :WG], lhsT=Ws, rhs=mean,
                             start=True, stop=True)
            uu = mds.tile([P, WG], f32, tag="uu")
            nc.scalar.activation(uu, tgps[:, :WG], AF.Sigmoid, scale=2.0)
            tg = mds.tile([P, WG], bf, tag="tg")
            nc.vector.tensor_scalar(tg, uu, 2.0, -1.0,
                                    op0=AluOpType.mult, op1=AluOpType.add)

            # ---- phase C: per-graph dots -> sigmoid'd row -> bcast -> weighted
            # dotsrow holds coef = sigmoid(2*dots) directly: the sigmoid is
            # fused into the PSUM->SBUF drain on ACT, and the K=1 broadcast
            # matmul replicates the finished coefs to all 128 partitions.
            dotsrow = drp.tile([1, WIN], bf, tag="dr")
            if probe == 3:      # timing probe: skip the dots stage
                nc.vector.memset(dotsrow, 0.0)
            else:
                # pack 2 graphs per PSUM bank when they fit (2*Sg <= 512)
                per = 2 if 2 * Sg <= 512 else 1
                for r0 in range(0, WG, per):
                    dps = dpp.tile([1, 512], f32, tag="dp")
                    npk = min(per, WG - r0)
                    for j in range(npk):
                        r = r0 + j
                        nc.tensor.matmul(dps[:, j * Sg : (j + 1) * Sg],
                                         lhsT=tg[:, r : r + 1],
                                         rhs=x2w[:, r * Sg : (r + 1) * Sg],
                                         start=True, stop=True)
                    nc.scalar.activation(
                        dotsrow[0:1, r0 * Sg : (r0 + npk) * Sg],
                        dps[:, : npk * Sg], AF.Sigmoid, scale=2.0)
            wt = wtp.tile([P, WIN], bf, tag="wt")
            k = 0
            while k < NC:
                w = 1024 if k + 1 < NC else 512
                c0 = k * 512
                cps = cpp.tile([P, 1024], f32, tag="cp")
                for j in range(w // 512):
                    nc.tensor.matmul(cps[:, 512 * j : 512 * (j + 1)],
                                     lhsT=onesb,
                                     rhs=dotsrow[0:1, c0 + 512 * j : c0 + 512 * (j + 1)],
                                     start=True, stop=True)
                nc.vector.tensor_tensor(wt[:, c0 : c0 + w],
                                        x2w[:, c0 : c0 + w], cps[:, :w],
                                        op=AluOpType.mult)
                k += w // 512
            nc.vector.reduce_sum(outacc[:, WG * g : WG * (g + 1)],
                                 wt.rearrange("p (r s) -> p r s", s=Sg),
                                 axis=mybir.AxisListType.X)

        nc.sync.dma_start(out=outd[:, :], in_=outacc)

    nc.compile()
    return nc


# ---------------------------------------------------------------- driver

def _make_in_maps(inputs):
    x = np.asarray(inputs["x"], _F32)
    batch = np.asarray(inputs["batch"]).astype(np.int64)
    fc_w1 = np.asarray(inputs["fc_w1"], _F32)
    fc_b1 = np.asarray(inputs["fc_b1"], _F32)
    fc_w2 = np.asarray(inputs["fc_w2"], _F32)
    fc_b2 = np.asarray(inputs["fc_b2"], _F32)
    W = np.asarray(inputs["W"], _F32)

    q8, step = _quantize(x)
    xT, ic, gid, S, off, TOT = _prep(x, batch, q8)
    pkb, pkf = _pack_params(fc_w1, fc_w2, W, fc_b1, fc_b2)
    in_maps = []
    for c in range(NCORES):
        in_maps.append({"x": xT[c], "pkb": pkb, "pkf": pkf, "ic": ic[c]})
    flags = (bool(np.abs(fc_b1).max() > 0), bool(np.abs(fc_b2).max() > 0))
    return in_maps, gid, S, step, flags


def _unshard(results, gid):
    out = np.zeros((G, D), dtype=np.float64)
    for c in range(NCORES):
        oc = np.asarray(results[c]["out"], _F32)    # [128, GPC]
        out[gid[c]] = 2.0 * oc.T.astype(np.float64)
    return out.astype(np.float32)


def _run(inputs, trace=False):
    import sys
    if "/opt/trn_rl_repo" not in sys.path:
        sys.path.insert(0, "/opt/trn_rl_repo")
    from concourse.bass_utils import run_bass_kernel_spmd

    in_maps, gid, S, step, (use_b1, use_b2) = _make_in_maps(inputs)
    nc = _build(S, step, use_b1=use_b1, use_b2=use_b2)
    res = run_bass_kernel_spmd(nc, in_maps, core_ids=list(range(NCORES)),
                               trace=trace)
    return _unshard(res.results, gid), res


def kernel(**inputs) -> np.ndarray:
    out, _ = _run(inputs, trace=False)
    return out


# ------------------------------------------------- bench (timing) harness

def _bench(inputs, iters=24):
    """Return (out, per_call_ns, single_ns) via steady-state async enqueue."""
    import sys, time
    if "/opt/trn_rl_repo" not in sys.path:
        sys.path.insert(0, "/opt/trn_rl_repo")
    import jax
    from jax.experimental.shard_map import shard_map
    from jax.sharding import Mesh, PartitionSpec
    from concourse import bass2jax, mybir
    from concourse.bass2jax import _bass_exec_p, partition_id_tensor

    bass2jax.install_neuronx_cc_hook()
    in_maps, gid, S, step, (use_b1, use_b2) = _make_in_maps(inputs)
    nc = _build(S, step, use_b1=use_b1, use_b2=use_b2)

    in_names, out_names, out_avals, zero_outs = [], [], [], []
    for alloc in nc.m.functions[0].allocations:
        if not isinstance(alloc, mybir.MemoryLocationSet):
            continue
        name = alloc.memorylocations[0].name
        if alloc.kind == "ExternalInput":
            if nc.partition_id_tensor is None or name != nc.partition_id_tensor.name:
                in_names.append(name)
        elif alloc.kind == "ExternalOutput":
            shape = tuple(alloc.tensor_shape)
            dtype = mybir.dt.np(alloc.dtype)
            out_names.append(name)
            out_avals.append(jax.core.ShapedArray(shape, dtype))
            zero_outs.append(np.zeros(shape, dtype))
    n_params = len(in_names)
    all_names = list(in_names) + out_names
    pname = nc.partition_id_tensor.name if nc.partition_id_tensor else None
    if pname is not None:
        all_names.append(pname)

    def _body(*args):
        operands = list(args)
        if pname is not None:
            operands.append(partition_id_tensor())
        return tuple(_bass_exec_p.bind(
            *operands, out_avals=tuple(out_avals), in_names=tuple(all_names),
            out_names=tuple(out_names), lowering_input_output_aliases=(),
            sim_require_finite=True, sim_require_nnan=True, nc=nc))

    devices = jax.devices()[:NCORES]
    mesh = Mesh(np.asarray(devices), ("core",))
    nio = n_params + len(out_names)
    fn = jax.jit(shard_map(_body, mesh=mesh,
                           in_specs=(PartitionSpec("core"),) * nio,
                           out_specs=(PartitionSpec("core"),) * len(out_names),
                           check_rep=False), keep_unused=True)
    concat_in = [np.concatenate([np.asarray(in_maps[c][nm])[None]
                                 for c in range(NCORES)], axis=0)
                 .reshape(-1, *np.asarray(in_maps[0][nm]).shape[1:])
                 for nm in in_names]
    concat_zero = [np.concatenate([z[None]] * NCORES, axis=0)
                   .reshape(-1, *z.shape[1:]) for z in zero_outs]
    dev_in = [jax.device_put(a) for a in concat_in + concat_zero]
    outs = fn(*dev_in)
    jax.block_until_ready(outs)
    t0 = time.perf_counter()
    outs = fn(*dev_in)
    jax.block_until_ready(outs)
    one = time.perf_counter() - t0
    t0 = time.perf_counter()
    last = None
    for _ in range(iters):
        last = fn(*dev_in)
    jax.block_until_ready(last)
    per = (time.perf_counter() - t0) / iters
    oarr = np.asarray(outs[0]).reshape(NCORES, P, GPC)
    out_full = _unshard([{"out": oarr[c]} for c in range(NCORES)], gid)
    return out_full, per * 1e9, one * 1e9
